# revision 21
# baseline (speedup 1.0000x reference)
"""DVQVAE (encoder -> VQ codebook argmin -> decoder) Trainium2 Bass kernel.

Strategy (8 NeuronCores, data-parallel over the 65536 tokens):
  - Each core processes 8192 tokens with all weights replicated.
  - All matmuls run as float32r (fp32 with 11-bit mantissa, full PE rate).
  - The decoder is evaluated once per CODEBOOK ENTRY (4096 rows), not per
    token: decoded rows are then fetched by index with indirect DMA
    (dma_gather), which removes ~80% of the decoder FLOPs.
  - Quantized rows are gathered from the raw fp32 codebook (bit-exact).
  - fp32r rounding can flip the VQ argmin when the top-2 score gap is tiny;
    the device also returns that gap and the host recomputes the ~1% of
    tokens whose gap is below a safety threshold in full fp32.

Layout notes: activations live feature-on-partition (xT/hT/eT) so layers
chain on the PE without transposes; x is transposed on-chip via the PE
transpose path. VQ scores are token-on-partition so the DVE max/max_index
top-8 instructions produce the argmin along the free axis.
"""
import sys

sys.path.insert(0, "/opt/trn_rl_repo")

import numpy as np
import concourse.bass as bass
import concourse.bacc as bacc
import concourse.mybir as mybir
import concourse.tile as tile
from concourse.bass_utils import run_bass_kernel_spmd

dt = mybir.dt
F32, F32R, U16, I16 = dt.float32, dt.float32r, dt.uint16, dt.int16
AF = mybir.ActivationFunctionType

N = 65536
NCORES = 8
NTOK = N // NCORES          # 8192 tokens per core
CHUNK = 256                 # tokens per main-loop chunk
NCHUNK = NTOK // CHUNK      # 32
NGRP = NTOK // 128          # 64 groups of 128 tokens
IN_D, HID_D, EMB_D, K = 1024, 2048, 256, 4096

GAP_THRESHOLD = 0.02        # host fixup flag: top-2 score gap below this


def r32r(a):
    """Round fp32 -> float32r bit pattern (11-bit mantissa, low 12 bits 0)."""
    u = np.ascontiguousarray(a, np.float32).view(np.uint32)
    return ((u + 0x800) & 0xFFFFF000).view(np.float32)


def build():
    nc = bacc.Bacc(None)

    x_p = nc.declare_dram_parameter("x", [NTOK, IN_D], F32, isOutput=False)
    W1_p = nc.declare_dram_parameter("W1r", [IN_D, HID_D], F32R, isOutput=False)
    W2_p = nc.declare_dram_parameter("W2r", [HID_D, EMB_D], F32R, isOutput=False)
    W3_p = nc.declare_dram_parameter("W3r", [EMB_D, HID_D], F32R, isOutput=False)
    W4_p = nc.declare_dram_parameter("W4r", [HID_D, IN_D], F32R, isOutput=False)
    cbT_p = nc.declare_dram_parameter("cbTr", [EMB_D, K], F32R, isOutput=False)
    cb_p = nc.declare_dram_parameter("cb", [K, EMB_D], F32, isOutput=False)
    cbsqh_p = nc.declare_dram_parameter("cbsqh", [128, K], F32, isOutput=False)
    b1_p = nc.declare_dram_parameter("b1l", [128, 16], F32, isOutput=False)
    b2_p = nc.declare_dram_parameter("b2l", [128, 2], F32, isOutput=False)
    b3_p = nc.declare_dram_parameter("b3l", [128, 16], F32, isOutput=False)
    b4_p = nc.declare_dram_parameter("b4r", [1, IN_D], F32R, isOutput=False)
    ones_p = nc.declare_dram_parameter("ones1", [1, 128], F32R, isOutput=False)
    id_p = nc.declare_dram_parameter("ident", [128, 128], F32, isOutput=False)

    quant_o = nc.declare_dram_parameter("quant", [NTOK, EMB_D], F32, isOutput=True)
    dec_o = nc.declare_dram_parameter("dec", [NTOK, IN_D], F32, isOutput=True)
    gaps_o = nc.declare_dram_parameter("gaps", [128, NGRP], F32, isOutput=True)
    idxs_o = nc.declare_dram_parameter("idxs", [128, NGRP], U16, isOutput=True)
    import os
    dbg = os.environ.get("BASSDBG") == "1"
    if dbg:
        dxT_o = nc.declare_dram_parameter("dxT", [128, 8, CHUNK], F32, isOutput=True)
        dhT_o = nc.declare_dram_parameter("dhT", [128, CHUNK], F32, isOutput=True)
        deT_o = nc.declare_dram_parameter("deT", [128, 2, CHUNK], F32, isOutput=True)
        dsc_o = nc.declare_dram_parameter("dsc", [128, K], F32, isOutput=True)

    dec_d = nc.dram_tensor("dec_d", [K, IN_D], F32)       # decoded-per-code table
    wd = nc.dram_tensor("wd_idx", [NCHUNK * CHUNK], U16)  # idx bounce buffer

    with tile.TileContext(nc) as tc:
        pconst = tc.tile_pool(name="pconst", bufs=1)
        with pconst as constp:
            cbT = constp.tile([128, 2, K], F32R)
            cbsq = constp.tile([128, K], F32)
            b1 = constp.tile([128, 16], F32)
            b2 = constp.tile([128, 2], F32)
            b3 = constp.tile([128, 16], F32)
            ident = constp.tile([128, 128], F32)
            for e in range(2):
                nc.sync.dma_start(out=cbT[:, e, :], in_=cbT_p[e * 128 : (e + 1) * 128, :])
            nc.sync.dma_start(out=cbsq, in_=cbsqh_p[:])
            nc.sync.dma_start(out=b1, in_=b1_p[:])
            nc.sync.dma_start(out=b2, in_=b2_p[:])
            nc.sync.dma_start(out=b3, in_=b3_p[:])
            nc.sync.dma_start(out=ident, in_=id_p[:])

            # ---- Phase 0: decoder table over the 4096 codebook rows ----
            with (
                tc.tile_pool(name="pw34", bufs=1) as w34p,
                tc.tile_pool(name="ptbl", bufs=1) as tblp,
                tc.tile_pool(name="pdec", bufs=2) as decp,
                tc.tile_pool(name="ps_h2", bufs=2, space="PSUM") as ps_h2,
                tc.tile_pool(name="ps_dec", bufs=2, space="PSUM") as ps_dec,
            ):
                W3 = w34p.tile([128, 2, HID_D], F32R)
                W4 = w34p.tile([128, 16, IN_D], F32R)
                b4 = w34p.tile([1, IN_D], F32R)
                ones = w34p.tile([1, 128], F32R)
                nc.sync.dma_start(out=b4, in_=b4_p[:])
                nc.sync.dma_start(out=ones, in_=ones_p[:])
                for k in range(2):
                    nc.sync.dma_start(out=W3[:, k, :], in_=W3_p[k * 128 : (k + 1) * 128, :])
                for k in range(16):
                    nc.sync.dma_start(out=W4[:, k, :], in_=W4_p[k * 128 : (k + 1) * 128, :])

                for cc in range(8):  # 512 codes per iteration
                    h2T = tblp.tile([128, 16, 512], F32R)
                    for m in range(16):
                        ph = ps_h2.tile([128, 512], F32)
                        for k in range(2):
                            nc.tensor.matmul(
                                ph,
                                W3[:, k, m * 128 : (m + 1) * 128],
                                cbT[:, k, cc * 512 : (cc + 1) * 512],
                                start=(k == 0),
                                stop=(k == 1),
                            )
                        nc.scalar.activation(h2T[:, m, :], ph, AF.Relu, bias=b3[:, m : m + 1])
                    for cs in range(4):  # 128-code subtiles -> code-major rows
                        decs = decp.tile([128, IN_D], F32)
                        for n in range(2):
                            pd = ps_dec.tile([128, 512], F32)
                            nc.tensor.matmul(
                                pd, ones, b4[:, n * 512 : (n + 1) * 512], start=True, stop=False
                            )
                            for k in range(16):
                                nc.tensor.matmul(
                                    pd,
                                    h2T[:, k, cs * 128 : (cs + 1) * 128],
                                    W4[:, k, n * 512 : (n + 1) * 512],
                                    start=False,
                                    stop=(k == 15),
                                )
                            nc.scalar.activation(decs[:, n * 512 : (n + 1) * 512], pd, AF.Copy)
                        nc.sync.dma_start(
                            out=dec_d[cc * 512 + cs * 128 : cc * 512 + (cs + 1) * 128, :],
                            in_=decs,
                        )

            # ---- Main loop: encoder + VQ + gathers, 256 tokens/chunk ----
            with (
                tc.tile_pool(name="pw12", bufs=1) as w12p,
                tc.tile_pool(name="pmain", bufs=1) as mp,
                tc.tile_pool(name="px", bufs=2) as xp,
                tc.tile_pool(name="pht", bufs=2) as htp,
                tc.tile_pool(name="pxt", bufs=1) as xtp,
                tc.tile_pool(name="pet", bufs=2) as etp,
                tc.tile_pool(name="pm8", bufs=4) as m8p,
                tc.tile_pool(name="pscore", bufs=2) as scp,
                tc.tile_pool(name="pidx", bufs=4) as idxp,
                tc.tile_pool(name="pwrap", bufs=4) as wrapp,
                tc.tile_pool(name="pgat", bufs=2) as gatp,
                tc.tile_pool(name="ps_mix", bufs=3, space="PSUM") as ps_mix,
                tc.tile_pool(name="ps_l1", bufs=2, space="PSUM") as ps_l1,
                tc.tile_pool(name="ps_et", bufs=1, space="PSUM") as ps_et,
            ):
                W1 = w12p.tile([128, 8, HID_D], F32R)
                W2 = w12p.tile([128, 16, EMB_D], F32R)
                for k in range(8):
                    nc.sync.dma_start(out=W1[:, k, :], in_=W1_p[k * 128 : (k + 1) * 128, :])
                for k in range(16):
                    nc.sync.dma_start(out=W2[:, k, :], in_=W2_p[k * 128 : (k + 1) * 128, :])

                gap = mp.tile([128, NGRP], F32)

                for c in range(NCHUNK):
                    xr = xp.tile([128, 2, IN_D], F32)
                    nc.sync.dma_start(
                        out=xr,
                        in_=x_p[c * CHUNK : (c + 1) * CHUNK, :].rearrange(
                            "(tp p) f -> p tp f", p=128
                        ),
                    )
                    xT = xtp.tile([128, 8, CHUNK], F32R, tag="xT")
                    for tp in range(2):
                        for k in range(8):
                            pt = ps_mix.tile([128, 512], F32, tag="mix")
                            nc.tensor.transpose(
                                pt[:, 0:128], xr[:, tp, k * 128 : (k + 1) * 128], ident
                            )
                            nc.vector.tensor_copy(
                                xT[:, k, tp * 128 : (tp + 1) * 128], pt[:, 0:128]
                            )
                    pe_ps0 = ps_et.tile([128, CHUNK], F32, tag="et0")
                    pe_ps1 = ps_et.tile([128, CHUNK], F32, tag="et1")
                    pe_ps = [pe_ps0, pe_ps1]
                    for m in range(16):
                        pl = ps_l1.tile([128, CHUNK], F32)
                        for k in range(8):
                            nc.tensor.matmul(
                                pl,
                                W1[:, k, m * 128 : (m + 1) * 128],
                                xT[:, k, :],
                                start=(k == 0),
                                stop=(k == 7),
                            )
                        hT = htp.tile([128, CHUNK], F32R)
                        nc.scalar.activation(hT, pl, AF.Relu, bias=b1[:, m : m + 1])
                        if dbg and c == 0 and m == 0:
                            nc.sync.dma_start(out=dhT_o[:], in_=hT.bitcast(F32))
                        for e in range(2):
                            nc.tensor.matmul(
                                pe_ps[e],
                                W2[:, m, e * 128 : (e + 1) * 128],
                                hT,
                                start=(m == 0),
                                stop=(m == 15),
                            )
                    eT = etp.tile([128, 2, CHUNK], F32R)
                    for e in range(2):
                        nc.scalar.activation(
                            eT[:, e, :], pe_ps[e], AF.Identity, bias=b2[:, e : e + 1]
                        )
                    if dbg and c == 0:
                        nc.sync.dma_start(out=dxT_o[:], in_=xT.bitcast(F32))
                        nc.sync.dma_start(out=deT_o[:], in_=eT.bitcast(F32))
                    # all 16 MM+subtract first so the DVE's long max/find ops
                    # never sit between the PE's VQ matmuls and their psum frees
                    scores = []
                    for j in range(2):  # two 128-token groups per chunk
                        score = scp.tile([128, K], F32, tag="score")
                        scores.append(score)
                        for cc in range(8):
                            pv = ps_mix.tile([128, 512], F32, tag="mix")
                            for e in range(2):
                                nc.tensor.matmul(
                                    pv,
                                    eT[:, e, j * 128 : (j + 1) * 128],
                                    cbT[:, e, cc * 512 : (cc + 1) * 512],
                                    start=(e == 0),
                                    stop=(e == 1),
                                )
                            nc.vector.tensor_sub(
                                score[:, cc * 512 : (cc + 1) * 512],
                                pv,
                                cbsq[:, cc * 512 : (cc + 1) * 512],
                            )
                    if dbg and c == 0:
                        nc.sync.dma_start(out=dsc_o[:], in_=scores[0])
                    wd_gp = wd[:].rearrange("(g p) -> g p", p=128)
                    for j in range(2):
                        g = 2 * c + j
                        m8 = m8p.tile([128, 8], F32)
                        i8 = m8p.tile([128, 8], U16, tag="i8")
                        nc.vector.max(m8, scores[j])
                        nc.vector.max_index(i8, m8, scores[j])
                        nc.vector.tensor_sub(
                            gap[:, g : g + 1], m8[:, 0:1], m8[:, 1:2]
                        )
                        idx16 = idxp.tile([128, 1], U16)
                        nc.vector.tensor_copy(idx16, i8[:, 0:1])
                        nc.scalar.dma_start(out=idxs_o[:, g : g + 1], in_=idx16)
                        # idx bounce: p-major dump, then wrapped+replicated read
                        nc.scalar.dma_start(out=wd_gp[g], in_=idx16[:, 0])
                        wrap = wrapp.tile([128, 8], U16)
                        nc.gpsimd.dma_start(
                            out=wrap,
                            in_=bass.AP(
                                tensor=wd,
                                offset=g * 128,
                                ap=[[0, 8], [8, 16], [1, 8]],
                            ),
                        )
                        # gather slot p' holds token g*128 + (p'%16)*8 + p'//16
                        decG = gatp.tile([128, 1, IN_D], F32)
                        nc.gpsimd.dma_gather(
                            out_ap=decG,
                            in_ap=dec_d[:],
                            idxs_ap=wrap.bitcast(I16),
                            num_idxs=128,
                            num_idxs_reg=128,
                            elem_size=IN_D,
                        )
                        nc.scalar.dma_start(
                            out=bass.AP(
                                tensor=dec_o,
                                offset=g * 128 * IN_D,
                                ap=[[IN_D, 8], [8 * IN_D, 16], [1, IN_D]],
                            ),
                            in_=decG,
                        )
                        cbG = gatp.tile([128, 1, EMB_D], F32, tag="cbG")
                        nc.gpsimd.dma_gather(
                            out_ap=cbG,
                            in_ap=cb_p[:],
                            idxs_ap=wrap.bitcast(I16),
                            num_idxs=128,
                            num_idxs_reg=128,
                            elem_size=EMB_D,
                        )
                        nc.scalar.dma_start(
                            out=bass.AP(
                                tensor=quant_o,
                                offset=g * 128 * EMB_D,
                                ap=[[EMB_D, 8], [8 * EMB_D, 16], [1, EMB_D]],
                            ),
                            in_=cbG,
                        )
                nc.scalar.dma_start(out=gaps_o[:], in_=gap)
    nc.finalize()
    return nc


_NC_CACHE = None


def _get_nc():
    global _NC_CACHE
    if _NC_CACHE is None:
        _NC_CACHE = build()
    return _NC_CACHE


def kernel(x, W1, b1, W2, b2, codebook, W3, b3, W4, b4):
    x = np.ascontiguousarray(np.asarray(x, np.float32))
    W1 = np.asarray(W1, np.float32)
    b1 = np.asarray(b1, np.float32)
    W2 = np.asarray(W2, np.float32)
    b2 = np.asarray(b2, np.float32)
    cb = np.ascontiguousarray(np.asarray(codebook, np.float32))
    W3 = np.asarray(W3, np.float32)
    b3 = np.asarray(b3, np.float32)
    W4 = np.asarray(W4, np.float32)
    b4 = np.asarray(b4, np.float32)

    cbsq = (cb.astype(np.float64) ** 2).sum(-1).astype(np.float32)
    common = dict(
        W1r=r32r(W1),
        W2r=r32r(W2),
        W3r=r32r(W3),
        W4r=r32r(W4),
        cbTr=r32r(np.ascontiguousarray(cb.T)),
        cb=cb,
        cbsqh=np.ascontiguousarray(np.broadcast_to(0.5 * cbsq, (128, K))),
        b1l=np.ascontiguousarray(b1.reshape(16, 128).T),
        b2l=np.ascontiguousarray(b2.reshape(2, 128).T),
        b3l=np.ascontiguousarray(b3.reshape(16, 128).T),
        b4r=r32r(b4)[None, :],
        ones1=np.ones((1, 128), np.float32),
        ident=np.eye(128, dtype=np.float32),
    )
    in_maps = [
        dict(common, x=x[i * NTOK : (i + 1) * NTOK]) for i in range(NCORES)
    ]
    nc = _get_nc()
    res = run_bass_kernel_spmd(nc, in_maps, core_ids=list(range(NCORES)))

    quant = np.concatenate([res.results[i]["quant"] for i in range(NCORES)], 0)
    dec = np.concatenate([res.results[i]["dec"] for i in range(NCORES)], 0)
    gaps = np.concatenate(
        [
            res.results[i]["gaps"].T.reshape(-1)  # [128, NGRP] -> token order
            for i in range(NCORES)
        ]
    )

    # Host fixup: recompute tokens whose top-2 VQ score gap is tiny enough
    # that fp32r rounding could have flipped the argmin.
    flagged = np.nonzero(gaps < GAP_THRESHOLD)[0]
    if flagged.size:
        xf = x[flagged]
        h = np.maximum(xf @ W1 + b1, 0.0)
        e = h @ W2 + b2
        d2 = (e * e).sum(-1, keepdims=True) - 2.0 * (e @ cb.T) + cbsq
        idx = np.argmin(d2, axis=-1)
        q = cb[idx]
        h2 = np.maximum(q @ W3 + b3, 0.0)
        df = h2 @ W4 + b4
        quant[flagged] = q
        dec[flagged] = df
    return quant, dec


# revision 22
# speedup vs baseline: 1.4551x; 1.4551x over previous
"""DVQVAE (encoder -> VQ codebook argmin -> decoder) Trainium2 Bass kernel.

Strategy (8 NeuronCores, data-parallel over the 65536 tokens):
  - Each core processes 8192 tokens with all weights replicated.
  - All matmuls run as float32r (fp32 with 11-bit mantissa, full PE rate).
  - The decoder is evaluated once per CODEBOOK ENTRY (4096 rows), not per
    token: decoded rows are then fetched by index with indirect DMA
    (dma_gather), which removes ~80% of the decoder FLOPs.
  - Quantized rows are gathered from the raw fp32 codebook (bit-exact).
  - fp32r rounding can flip the VQ argmin when the top-2 score gap is tiny;
    the device also returns that gap and the host recomputes the ~1% of
    tokens whose gap is below a safety threshold in full fp32.

Layout notes: activations live feature-on-partition (xT/hT/eT) so layers
chain on the PE without transposes; x is transposed on-chip via the PE
transpose path. VQ scores are token-on-partition so the DVE max/max_index
top-8 instructions produce the argmin along the free axis.
"""
import sys

sys.path.insert(0, "/opt/trn_rl_repo")

import numpy as np
import concourse.bass as bass
import concourse.bacc as bacc
import concourse.mybir as mybir
import concourse.tile as tile
from concourse.bass_utils import run_bass_kernel_spmd

dt = mybir.dt
F32, F32R, U16, I16 = dt.float32, dt.float32r, dt.uint16, dt.int16
AF = mybir.ActivationFunctionType

N = 65536
NCORES = 8
NTOK = N // NCORES          # 8192 tokens per core
CHUNK = 256                 # tokens per main-loop chunk
NCHUNK = NTOK // CHUNK      # 32
NGRP = NTOK // 128          # 64 groups of 128 tokens
IN_D, HID_D, EMB_D, K = 1024, 2048, 256, 4096

GAP_THRESHOLD = 0.02        # host fixup flag: top-2 score gap below this


def r32r(a):
    """Round fp32 -> float32r bit pattern (11-bit mantissa, low 12 bits 0)."""
    u = np.ascontiguousarray(a, np.float32).view(np.uint32)
    return ((u + 0x800) & 0xFFFFF000).view(np.float32)


def build():
    nc = bacc.Bacc(None)

    x_p = nc.declare_dram_parameter("x", [NTOK, IN_D], F32, isOutput=False)
    W1_p = nc.declare_dram_parameter("W1r", [IN_D, HID_D], F32R, isOutput=False)
    W2_p = nc.declare_dram_parameter("W2r", [HID_D, EMB_D], F32R, isOutput=False)
    W3_p = nc.declare_dram_parameter("W3r", [EMB_D, HID_D], F32R, isOutput=False)
    W4_p = nc.declare_dram_parameter("W4r", [HID_D, IN_D], F32R, isOutput=False)
    cbT_p = nc.declare_dram_parameter("cbTr", [EMB_D, K], F32R, isOutput=False)
    cb_p = nc.declare_dram_parameter("cb", [K, EMB_D], F32, isOutput=False)
    cbsqh_p = nc.declare_dram_parameter("cbsqh", [128, K], F32, isOutput=False)
    b1_p = nc.declare_dram_parameter("b1l", [128, 16], F32, isOutput=False)
    b2_p = nc.declare_dram_parameter("b2l", [128, 2], F32, isOutput=False)
    b3_p = nc.declare_dram_parameter("b3l", [128, 16], F32, isOutput=False)
    b4_p = nc.declare_dram_parameter("b4r", [1, IN_D], F32R, isOutput=False)
    ones_p = nc.declare_dram_parameter("ones1", [1, 128], F32R, isOutput=False)
    id_p = nc.declare_dram_parameter("ident", [128, 128], F32, isOutput=False)

    quant_o = nc.declare_dram_parameter("quant", [NTOK, EMB_D], F32, isOutput=True)
    dec_o = nc.declare_dram_parameter("dec", [NTOK, IN_D], F32, isOutput=True)
    gaps_o = nc.declare_dram_parameter("gaps", [128, NGRP], F32, isOutput=True)
    idxs_o = nc.declare_dram_parameter("idxs", [128, NGRP], U16, isOutput=True)
    import os
    dbg = os.environ.get("BASSDBG") == "1"
    if dbg:
        dxT_o = nc.declare_dram_parameter("dxT", [128, 8, CHUNK], F32, isOutput=True)
        dhT_o = nc.declare_dram_parameter("dhT", [128, CHUNK], F32, isOutput=True)
        deT_o = nc.declare_dram_parameter("deT", [128, 2, CHUNK], F32, isOutput=True)
        dsc_o = nc.declare_dram_parameter("dsc", [128, K], F32, isOutput=True)

    dec_d = nc.dram_tensor("dec_d", [K, IN_D], F32)       # decoded-per-code table
    wd = nc.dram_tensor("wd_idx", [NCHUNK * CHUNK], U16)  # idx bounce buffer

    with tile.TileContext(nc) as tc:
        pconst = tc.tile_pool(name="pconst", bufs=1)
        with pconst as constp:
            cbT = constp.tile([128, 2, K], F32R)
            cbsq = constp.tile([128, K], F32)
            b1 = constp.tile([128, 16], F32)
            b2 = constp.tile([128, 2], F32)
            b3 = constp.tile([128, 16], F32)
            ident = constp.tile([128, 128], F32)
            for e in range(2):
                nc.sync.dma_start(out=cbT[:, e, :], in_=cbT_p[e * 128 : (e + 1) * 128, :])
            nc.sync.dma_start(out=cbsq, in_=cbsqh_p[:])
            nc.sync.dma_start(out=b1, in_=b1_p[:])
            nc.sync.dma_start(out=b2, in_=b2_p[:])
            nc.sync.dma_start(out=b3, in_=b3_p[:])
            nc.sync.dma_start(out=ident, in_=id_p[:])

            # ---- Phase 0: decoder table over the 4096 codebook rows ----
            with (
                tc.tile_pool(name="pw34", bufs=1) as w34p,
                tc.tile_pool(name="ptbl", bufs=1) as tblp,
                tc.tile_pool(name="pdec", bufs=2) as decp,
                tc.tile_pool(name="ps_h2", bufs=2, space="PSUM") as ps_h2,
                tc.tile_pool(name="ps_dec", bufs=2, space="PSUM") as ps_dec,
            ):
                W3 = w34p.tile([128, 2, HID_D], F32R)
                W4 = w34p.tile([128, 16, IN_D], F32R)
                b4 = w34p.tile([1, IN_D], F32R)
                ones = w34p.tile([1, 128], F32R)
                nc.sync.dma_start(out=b4, in_=b4_p[:])
                nc.sync.dma_start(out=ones, in_=ones_p[:])
                for k in range(2):
                    nc.sync.dma_start(out=W3[:, k, :], in_=W3_p[k * 128 : (k + 1) * 128, :])
                for k in range(16):
                    nc.sync.dma_start(out=W4[:, k, :], in_=W4_p[k * 128 : (k + 1) * 128, :])

                for cc in range(8):  # 512 codes per iteration
                    h2T = tblp.tile([128, 16, 512], F32R)
                    for m in range(16):
                        ph = ps_h2.tile([128, 512], F32)
                        for k in range(2):
                            nc.tensor.matmul(
                                ph,
                                W3[:, k, m * 128 : (m + 1) * 128],
                                cbT[:, k, cc * 512 : (cc + 1) * 512],
                                start=(k == 0),
                                stop=(k == 1),
                            )
                        nc.scalar.activation(h2T[:, m, :], ph, AF.Relu, bias=b3[:, m : m + 1])
                    for cs in range(4):  # 128-code subtiles -> code-major rows
                        decs = decp.tile([128, IN_D], F32)
                        for n in range(2):
                            pd = ps_dec.tile([128, 512], F32)
                            nc.tensor.matmul(
                                pd, ones, b4[:, n * 512 : (n + 1) * 512], start=True, stop=False
                            )
                            for k in range(16):
                                nc.tensor.matmul(
                                    pd,
                                    h2T[:, k, cs * 128 : (cs + 1) * 128],
                                    W4[:, k, n * 512 : (n + 1) * 512],
                                    start=False,
                                    stop=(k == 15),
                                )
                            nc.scalar.activation(decs[:, n * 512 : (n + 1) * 512], pd, AF.Copy)
                        nc.sync.dma_start(
                            out=dec_d[cc * 512 + cs * 128 : cc * 512 + (cs + 1) * 128, :],
                            in_=decs,
                        )

            # ---- Main loop: encoder + VQ + gathers, 256 tokens/chunk ----
            with (
                tc.tile_pool(name="pw12", bufs=1) as w12p,
                tc.tile_pool(name="pmain", bufs=1) as mp,
                tc.tile_pool(name="px", bufs=2) as xp,
                tc.tile_pool(name="pht", bufs=2) as htp,
                tc.tile_pool(name="pxt", bufs=1) as xtp,
                tc.tile_pool(name="pet", bufs=2) as etp,
                tc.tile_pool(name="pm8", bufs=4) as m8p,
                tc.tile_pool(name="pscore", bufs=2) as scp,
                tc.tile_pool(name="pidx", bufs=4) as idxp,
                tc.tile_pool(name="pwrap", bufs=4) as wrapp,
                tc.tile_pool(name="pgat", bufs=2) as gatp,
                tc.tile_pool(name="ps_mix", bufs=3, space="PSUM") as ps_mix,
                tc.tile_pool(name="ps_l1", bufs=2, space="PSUM") as ps_l1,
                tc.tile_pool(name="ps_et", bufs=1, space="PSUM") as ps_et,
            ):
                W1 = w12p.tile([128, 8, HID_D], F32R)
                W2 = w12p.tile([128, 16, EMB_D], F32R)
                for k in range(8):
                    nc.sync.dma_start(out=W1[:, k, :], in_=W1_p[k * 128 : (k + 1) * 128, :])
                for k in range(16):
                    nc.sync.dma_start(out=W2[:, k, :], in_=W2_p[k * 128 : (k + 1) * 128, :])

                gap = mp.tile([128, NGRP], F32)

                for c in range(NCHUNK):
                    xr = xp.tile([128, 2, IN_D], F32)
                    nc.sync.dma_start(
                        out=xr,
                        in_=x_p[c * CHUNK : (c + 1) * CHUNK, :].rearrange(
                            "(tp p) f -> p tp f", p=128
                        ),
                    )
                    xT = xtp.tile([128, 8, CHUNK], F32R, tag="xT")
                    for tp in range(2):
                        for k in range(8):
                            pt = ps_mix.tile([128, 512], F32, tag="mix")
                            nc.tensor.transpose(
                                pt[:, 0:128], xr[:, tp, k * 128 : (k + 1) * 128], ident
                            )
                            nc.vector.tensor_copy(
                                xT[:, k, tp * 128 : (tp + 1) * 128], pt[:, 0:128]
                            )
                    pe_ps0 = ps_et.tile([128, CHUNK], F32, tag="et0")
                    pe_ps1 = ps_et.tile([128, CHUNK], F32, tag="et1")
                    pe_ps = [pe_ps0, pe_ps1]
                    for m in range(16):
                        pl = ps_l1.tile([128, CHUNK], F32)
                        for k in range(8):
                            nc.tensor.matmul(
                                pl,
                                W1[:, k, m * 128 : (m + 1) * 128],
                                xT[:, k, :],
                                start=(k == 0),
                                stop=(k == 7),
                            )
                        hT = htp.tile([128, CHUNK], F32R)
                        nc.scalar.activation(hT, pl, AF.Relu, bias=b1[:, m : m + 1])
                        if dbg and c == 0 and m == 0:
                            nc.sync.dma_start(out=dhT_o[:], in_=hT.bitcast(F32))
                        for e in range(2):
                            nc.tensor.matmul(
                                pe_ps[e],
                                W2[:, m, e * 128 : (e + 1) * 128],
                                hT,
                                start=(m == 0),
                                stop=(m == 15),
                            )
                    eT = etp.tile([128, 2, CHUNK], F32R)
                    for e in range(2):
                        nc.scalar.activation(
                            eT[:, e, :], pe_ps[e], AF.Identity, bias=b2[:, e : e + 1]
                        )
                    if dbg and c == 0:
                        nc.sync.dma_start(out=dxT_o[:], in_=xT.bitcast(F32))
                        nc.sync.dma_start(out=deT_o[:], in_=eT.bitcast(F32))
                    # all 16 MM+subtract first so the DVE's long max/find ops
                    # never sit between the PE's VQ matmuls and their psum frees
                    scores = []
                    for j in range(2):  # two 128-token groups per chunk
                        score = scp.tile([128, K], F32, tag="score")
                        scores.append(score)
                        for cc in range(8):
                            pv = ps_mix.tile([128, 512], F32, tag="mix")
                            for e in range(2):
                                nc.tensor.matmul(
                                    pv,
                                    eT[:, e, j * 128 : (j + 1) * 128],
                                    cbT[:, e, cc * 512 : (cc + 1) * 512],
                                    start=(e == 0),
                                    stop=(e == 1),
                                )
                            nc.vector.tensor_sub(
                                score[:, cc * 512 : (cc + 1) * 512],
                                pv,
                                cbsq[:, cc * 512 : (cc + 1) * 512],
                            )
                    if dbg and c == 0:
                        nc.sync.dma_start(out=dsc_o[:], in_=scores[0])
                    wd_gp = wd[:].rearrange("(g p) -> g p", p=128)
                    for j in range(2):
                        g = 2 * c + j
                        m8 = m8p.tile([128, 8], F32)
                        i8 = m8p.tile([128, 8], U16, tag="i8")
                        nc.vector.max(m8, scores[j])
                        nc.vector.max_index(i8, m8, scores[j])
                        nc.vector.tensor_sub(
                            gap[:, g : g + 1], m8[:, 0:1], m8[:, 1:2]
                        )
                        idx16 = idxp.tile([128, 1], U16)
                        nc.vector.tensor_copy(idx16, i8[:, 0:1])
                        nc.gpsimd.dma_start(out=idxs_o[:, g : g + 1], in_=idx16)
                        # idx bounce: p-major dump, then wrapped+replicated read
                        nc.gpsimd.dma_start(out=wd_gp[g], in_=idx16[:, 0])
                        wrap = wrapp.tile([128, 8], U16)
                        nc.gpsimd.dma_start(
                            out=wrap,
                            in_=bass.AP(
                                tensor=wd,
                                offset=g * 128,
                                ap=[[0, 8], [8, 16], [1, 8]],
                            ),
                        )
                        # gather slot p' holds token g*128 + (p'%16)*8 + p'//16
                        decG = gatp.tile([128, 1, IN_D], F32)
                        nc.gpsimd.dma_gather(
                            out_ap=decG,
                            in_ap=dec_d[:],
                            idxs_ap=wrap.bitcast(I16),
                            num_idxs=128,
                            num_idxs_reg=128,
                            elem_size=IN_D,
                        )
                        nc.gpsimd.dma_start(
                            out=bass.AP(
                                tensor=dec_o,
                                offset=g * 128 * IN_D,
                                ap=[[IN_D, 8], [8 * IN_D, 16], [1, IN_D]],
                            ),
                            in_=decG,
                        )
                        cbG = gatp.tile([128, 1, EMB_D], F32, tag="cbG")
                        nc.gpsimd.dma_gather(
                            out_ap=cbG,
                            in_ap=cb_p[:],
                            idxs_ap=wrap.bitcast(I16),
                            num_idxs=128,
                            num_idxs_reg=128,
                            elem_size=EMB_D,
                        )
                        nc.gpsimd.dma_start(
                            out=bass.AP(
                                tensor=quant_o,
                                offset=g * 128 * EMB_D,
                                ap=[[EMB_D, 8], [8 * EMB_D, 16], [1, EMB_D]],
                            ),
                            in_=cbG,
                        )
                nc.gpsimd.dma_start(out=gaps_o[:], in_=gap)
    nc.finalize()
    return nc


_NC_CACHE = None


def _get_nc():
    global _NC_CACHE
    if _NC_CACHE is None:
        _NC_CACHE = build()
    return _NC_CACHE


def kernel(x, W1, b1, W2, b2, codebook, W3, b3, W4, b4):
    x = np.ascontiguousarray(np.asarray(x, np.float32))
    W1 = np.asarray(W1, np.float32)
    b1 = np.asarray(b1, np.float32)
    W2 = np.asarray(W2, np.float32)
    b2 = np.asarray(b2, np.float32)
    cb = np.ascontiguousarray(np.asarray(codebook, np.float32))
    W3 = np.asarray(W3, np.float32)
    b3 = np.asarray(b3, np.float32)
    W4 = np.asarray(W4, np.float32)
    b4 = np.asarray(b4, np.float32)

    cbsq = (cb.astype(np.float64) ** 2).sum(-1).astype(np.float32)
    common = dict(
        W1r=r32r(W1),
        W2r=r32r(W2),
        W3r=r32r(W3),
        W4r=r32r(W4),
        cbTr=r32r(np.ascontiguousarray(cb.T)),
        cb=cb,
        cbsqh=np.ascontiguousarray(np.broadcast_to(0.5 * cbsq, (128, K))),
        b1l=np.ascontiguousarray(b1.reshape(16, 128).T),
        b2l=np.ascontiguousarray(b2.reshape(2, 128).T),
        b3l=np.ascontiguousarray(b3.reshape(16, 128).T),
        b4r=r32r(b4)[None, :],
        ones1=np.ones((1, 128), np.float32),
        ident=np.eye(128, dtype=np.float32),
    )
    in_maps = [
        dict(common, x=x[i * NTOK : (i + 1) * NTOK]) for i in range(NCORES)
    ]
    nc = _get_nc()
    res = run_bass_kernel_spmd(nc, in_maps, core_ids=list(range(NCORES)))

    quant = np.concatenate([res.results[i]["quant"] for i in range(NCORES)], 0)
    dec = np.concatenate([res.results[i]["dec"] for i in range(NCORES)], 0)
    gaps = np.concatenate(
        [
            res.results[i]["gaps"].T.reshape(-1)  # [128, NGRP] -> token order
            for i in range(NCORES)
        ]
    )

    # Host fixup: recompute tokens whose top-2 VQ score gap is tiny enough
    # that fp32r rounding could have flipped the argmin.
    flagged = np.nonzero(gaps < GAP_THRESHOLD)[0]
    if flagged.size:
        xf = x[flagged]
        h = np.maximum(xf @ W1 + b1, 0.0)
        e = h @ W2 + b2
        d2 = (e * e).sum(-1, keepdims=True) - 2.0 * (e @ cb.T) + cbsq
        idx = np.argmin(d2, axis=-1)
        q = cb[idx]
        h2 = np.maximum(q @ W3 + b3, 0.0)
        df = h2 @ W4 + b4
        quant[flagged] = q
        dec[flagged] = df
    return quant, dec


# revision 23
# speedup vs baseline: 1.7083x; 1.1740x over previous
"""DVQVAE (encoder -> VQ codebook argmin -> decoder) Trainium2 Bass kernel.

Strategy (8 NeuronCores, data-parallel over the 65536 tokens):
  - Each core processes 8192 tokens with all weights replicated.
  - All matmuls run as float32r (fp32 with 11-bit mantissa, full PE rate).
  - The decoder is evaluated once per CODEBOOK ENTRY (4096 rows), not per
    token: decoded rows are then fetched by index with indirect DMA
    (dma_gather), which removes ~80% of the decoder FLOPs.
  - Quantized rows are gathered from the raw fp32 codebook (bit-exact).
  - fp32r rounding can flip the VQ argmin when the top-2 score gap is tiny;
    the device also returns that gap and the host recomputes the ~1% of
    tokens whose gap is below a safety threshold in full fp32.

Layout notes: activations live feature-on-partition (xT/hT/eT) so layers
chain on the PE without transposes; x is transposed on-chip via the PE
transpose path. VQ scores are token-on-partition so the DVE max/max_index
top-8 instructions produce the argmin along the free axis.
"""
import sys

sys.path.insert(0, "/opt/trn_rl_repo")

import numpy as np
import concourse.bass as bass
import concourse.bacc as bacc
import concourse.mybir as mybir
import concourse.tile as tile
from concourse.bass_utils import run_bass_kernel_spmd

dt = mybir.dt
F32, F32R, U16, I16 = dt.float32, dt.float32r, dt.uint16, dt.int16
AF = mybir.ActivationFunctionType

N = 65536
NCORES = 8
NTOK = N // NCORES          # 8192 tokens per core
CHUNK = 256                 # tokens per main-loop chunk
NCHUNK = NTOK // CHUNK      # 32
NGRP = NTOK // 128          # 64 groups of 128 tokens
IN_D, HID_D, EMB_D, K = 1024, 2048, 256, 4096

GAP_THRESHOLD = 0.02        # host fixup flag: top-2 score gap below this


def r32r(a):
    """Round fp32 -> float32r bit pattern (11-bit mantissa, low 12 bits 0)."""
    u = np.ascontiguousarray(a, np.float32).view(np.uint32)
    return ((u + 0x800) & 0xFFFFF000).view(np.float32)


def build():
    nc = bacc.Bacc(None)

    x_p = nc.declare_dram_parameter("x", [NTOK, IN_D], F32, isOutput=False)
    W1_p = nc.declare_dram_parameter("W1r", [IN_D, HID_D], F32R, isOutput=False)
    W2_p = nc.declare_dram_parameter("W2r", [HID_D, EMB_D], F32R, isOutput=False)
    W3_p = nc.declare_dram_parameter("W3r", [EMB_D, HID_D], F32R, isOutput=False)
    W4_p = nc.declare_dram_parameter("W4r", [HID_D, IN_D], F32R, isOutput=False)
    cbT_p = nc.declare_dram_parameter("cbTr", [EMB_D, K], F32R, isOutput=False)
    cb_p = nc.declare_dram_parameter("cb", [K, EMB_D], F32, isOutput=False)
    cbsqh_p = nc.declare_dram_parameter("cbsqh", [128, K], F32, isOutput=False)
    b1_p = nc.declare_dram_parameter("b1l", [128, 16], F32, isOutput=False)
    b2_p = nc.declare_dram_parameter("b2l", [128, 2], F32, isOutput=False)
    b3_p = nc.declare_dram_parameter("b3l", [128, 16], F32, isOutput=False)
    b4_p = nc.declare_dram_parameter("b4r", [1, IN_D], F32R, isOutput=False)
    ones_p = nc.declare_dram_parameter("ones1", [1, 128], F32R, isOutput=False)
    id_p = nc.declare_dram_parameter("ident", [128, 128], F32, isOutput=False)

    quant_o = nc.declare_dram_parameter("quant", [NTOK, EMB_D], F32, isOutput=True)
    dec_o = nc.declare_dram_parameter("dec", [NTOK, IN_D], F32, isOutput=True)
    gaps_o = nc.declare_dram_parameter("gaps", [128, NGRP], F32, isOutput=True)
    idxs_o = nc.declare_dram_parameter("idxs", [128, NGRP], U16, isOutput=True)
    import os
    dbg = os.environ.get("BASSDBG") == "1"
    if dbg:
        dxT_o = nc.declare_dram_parameter("dxT", [128, 8, CHUNK], F32, isOutput=True)
        dhT_o = nc.declare_dram_parameter("dhT", [128, CHUNK], F32, isOutput=True)
        deT_o = nc.declare_dram_parameter("deT", [128, 2, CHUNK], F32, isOutput=True)
        dsc_o = nc.declare_dram_parameter("dsc", [128, K], F32, isOutput=True)

    dec_d = nc.dram_tensor("dec_d", [K, IN_D], F32)       # decoded-per-code table
    wd = nc.dram_tensor("wd_idx", [NCHUNK * CHUNK], U16)  # idx bounce buffer

    with tile.TileContext(nc) as tc:
        pconst = tc.tile_pool(name="pconst", bufs=1)
        with pconst as constp:
            cbT = constp.tile([128, 2, K], F32R)
            cbsq = constp.tile([128, K], F32)
            b1 = constp.tile([128, 16], F32)
            b2 = constp.tile([128, 2], F32)
            b3 = constp.tile([128, 16], F32)
            ident = constp.tile([128, 128], F32)
            for e in range(2):
                nc.sync.dma_start(out=cbT[:, e, :], in_=cbT_p[e * 128 : (e + 1) * 128, :])
            nc.sync.dma_start(out=cbsq, in_=cbsqh_p[:])
            nc.sync.dma_start(out=b1, in_=b1_p[:])
            nc.sync.dma_start(out=b2, in_=b2_p[:])
            nc.sync.dma_start(out=b3, in_=b3_p[:])
            nc.sync.dma_start(out=ident, in_=id_p[:])

            # ---- Phase 0: decoder table over the 4096 codebook rows ----
            with (
                tc.tile_pool(name="pw34", bufs=1) as w34p,
                tc.tile_pool(name="ptbl", bufs=1) as tblp,
                tc.tile_pool(name="pdec", bufs=2) as decp,
                tc.tile_pool(name="ps_h2", bufs=2, space="PSUM") as ps_h2,
                tc.tile_pool(name="ps_dec", bufs=2, space="PSUM") as ps_dec,
            ):
                W3 = w34p.tile([128, 2, HID_D], F32R)
                W4 = w34p.tile([128, 16, IN_D], F32R)
                b4 = w34p.tile([1, IN_D], F32R)
                ones = w34p.tile([1, 128], F32R)
                nc.sync.dma_start(out=b4, in_=b4_p[:])
                nc.sync.dma_start(out=ones, in_=ones_p[:])
                for k in range(2):
                    nc.sync.dma_start(out=W3[:, k, :], in_=W3_p[k * 128 : (k + 1) * 128, :])
                for k in range(16):
                    nc.sync.dma_start(out=W4[:, k, :], in_=W4_p[k * 128 : (k + 1) * 128, :])

                for cc in range(8):  # 512 codes per iteration
                    h2T = tblp.tile([128, 16, 512], F32R)
                    for m in range(16):
                        ph = ps_h2.tile([128, 512], F32)
                        for k in range(2):
                            nc.tensor.matmul(
                                ph,
                                W3[:, k, m * 128 : (m + 1) * 128],
                                cbT[:, k, cc * 512 : (cc + 1) * 512],
                                start=(k == 0),
                                stop=(k == 1),
                            )
                        nc.scalar.activation(h2T[:, m, :], ph, AF.Relu, bias=b3[:, m : m + 1])
                    for cs in range(4):  # 128-code subtiles -> code-major rows
                        decs = decp.tile([128, IN_D], F32)
                        for n in range(2):
                            pd = ps_dec.tile([128, 512], F32)
                            nc.tensor.matmul(
                                pd, ones, b4[:, n * 512 : (n + 1) * 512], start=True, stop=False
                            )
                            for k in range(16):
                                nc.tensor.matmul(
                                    pd,
                                    h2T[:, k, cs * 128 : (cs + 1) * 128],
                                    W4[:, k, n * 512 : (n + 1) * 512],
                                    start=False,
                                    stop=(k == 15),
                                )
                            nc.scalar.activation(decs[:, n * 512 : (n + 1) * 512], pd, AF.Copy)
                        nc.sync.dma_start(
                            out=dec_d[cc * 512 + cs * 128 : cc * 512 + (cs + 1) * 128, :],
                            in_=decs,
                        )

            # ---- Main loop: encoder + VQ + gathers, 256 tokens/chunk ----
            with (
                tc.tile_pool(name="pw12", bufs=1) as w12p,
                tc.tile_pool(name="pmain", bufs=1) as mp,
                tc.tile_pool(name="px", bufs=2) as xp,
                tc.tile_pool(name="pht", bufs=2) as htp,
                tc.tile_pool(name="pxt", bufs=1) as xtp,
                tc.tile_pool(name="pet", bufs=2) as etp,
                tc.tile_pool(name="pm8", bufs=4) as m8p,
                tc.tile_pool(name="pscore", bufs=2) as scp,
                tc.tile_pool(name="pidx", bufs=4) as idxp,
                tc.tile_pool(name="pwrap", bufs=4) as wrapp,
                tc.tile_pool(name="pgat", bufs=2) as gatp,
                tc.tile_pool(name="ps_mix", bufs=3, space="PSUM") as ps_mix,
                tc.tile_pool(name="ps_l1", bufs=2, space="PSUM") as ps_l1,
                tc.tile_pool(name="ps_et", bufs=1, space="PSUM") as ps_et,
            ):
                W1 = w12p.tile([128, 8, HID_D], F32R)
                W2 = w12p.tile([128, 16, EMB_D], F32R)
                for k in range(8):
                    nc.sync.dma_start(out=W1[:, k, :], in_=W1_p[k * 128 : (k + 1) * 128, :])
                for k in range(16):
                    nc.sync.dma_start(out=W2[:, k, :], in_=W2_p[k * 128 : (k + 1) * 128, :])

                gap = mp.tile([128, NGRP], F32)

                for c in range(NCHUNK):
                    xr = xp.tile([128, 2, IN_D], F32)
                    nc.sync.dma_start(
                        out=xr,
                        in_=x_p[c * CHUNK : (c + 1) * CHUNK, :].rearrange(
                            "(tp p) f -> p tp f", p=128
                        ),
                    )
                    xT = xtp.tile([128, 8, CHUNK], F32R, tag="xT")
                    for tp in range(2):
                        for kq in range(2):  # 4 packed transposes per PSUM bank
                            pt = ps_mix.tile([128, 4, 128], F32, tag="mix")
                            for ki in range(4):
                                k = kq * 4 + ki
                                nc.tensor.transpose(
                                    pt[:, ki, :], xr[:, tp, k * 128 : (k + 1) * 128], ident
                                )
                            nc.scalar.activation(
                                xT[:, kq * 4 : (kq + 1) * 4, tp * 128 : (tp + 1) * 128],
                                pt,
                                AF.Copy,
                            )
                    pe_ps0 = ps_et.tile([128, CHUNK], F32, tag="et0")
                    pe_ps1 = ps_et.tile([128, CHUNK], F32, tag="et1")
                    pe_ps = [pe_ps0, pe_ps1]
                    for m in range(16):
                        pl = ps_l1.tile([128, CHUNK], F32)
                        for k in range(8):
                            nc.tensor.matmul(
                                pl,
                                W1[:, k, m * 128 : (m + 1) * 128],
                                xT[:, k, :],
                                start=(k == 0),
                                stop=(k == 7),
                            )
                        hT = htp.tile([128, CHUNK], F32R)
                        nc.scalar.activation(hT, pl, AF.Relu, bias=b1[:, m : m + 1])
                        if dbg and c == 0 and m == 0:
                            nc.sync.dma_start(out=dhT_o[:], in_=hT.bitcast(F32))
                        for e in range(2):
                            nc.tensor.matmul(
                                pe_ps[e],
                                W2[:, m, e * 128 : (e + 1) * 128],
                                hT,
                                start=(m == 0),
                                stop=(m == 15),
                            )
                    eT = etp.tile([128, 2, CHUNK], F32R)
                    for e in range(2):
                        nc.scalar.activation(
                            eT[:, e, :], pe_ps[e], AF.Identity, bias=b2[:, e : e + 1]
                        )
                    if dbg and c == 0:
                        nc.sync.dma_start(out=dxT_o[:], in_=xT.bitcast(F32))
                        nc.sync.dma_start(out=deT_o[:], in_=eT.bitcast(F32))
                    # all 16 MM+subtract first so the DVE's long max/find ops
                    # never sit between the PE's VQ matmuls and their psum frees
                    scores = []
                    for j in range(2):  # two 128-token groups per chunk
                        score = scp.tile([128, K], F32, tag="score")
                        scores.append(score)
                        for cc in range(8):
                            pv = ps_mix.tile([128, 512], F32, tag="mix")
                            for e in range(2):
                                nc.tensor.matmul(
                                    pv,
                                    eT[:, e, j * 128 : (j + 1) * 128],
                                    cbT[:, e, cc * 512 : (cc + 1) * 512],
                                    start=(e == 0),
                                    stop=(e == 1),
                                )
                            nc.vector.tensor_sub(
                                score[:, cc * 512 : (cc + 1) * 512],
                                pv,
                                cbsq[:, cc * 512 : (cc + 1) * 512],
                            )
                    if dbg and c == 0:
                        nc.sync.dma_start(out=dsc_o[:], in_=scores[0])
                    wd_gp = wd[:].rearrange("(g p) -> g p", p=128)
                    for j in range(2):
                        g = 2 * c + j
                        m8 = m8p.tile([128, 8], F32)
                        i8 = m8p.tile([128, 8], U16, tag="i8")
                        nc.vector.max(m8, scores[j])
                        nc.vector.max_index(i8, m8, scores[j])
                        nc.vector.tensor_sub(
                            gap[:, g : g + 1], m8[:, 0:1], m8[:, 1:2]
                        )
                        idx16 = idxp.tile([128, 1], U16)
                        nc.vector.tensor_copy(idx16, i8[:, 0:1])
                        nc.gpsimd.dma_start(out=idxs_o[:, g : g + 1], in_=idx16)
                        # idx bounce: p-major dump, then wrapped+replicated read
                        nc.gpsimd.dma_start(out=wd_gp[g], in_=idx16[:, 0])
                        wrap = wrapp.tile([128, 8], U16)
                        nc.gpsimd.dma_start(
                            out=wrap,
                            in_=bass.AP(
                                tensor=wd,
                                offset=g * 128,
                                ap=[[0, 8], [8, 16], [1, 8]],
                            ),
                        )
                        # gather slot p' holds token g*128 + (p'%16)*8 + p'//16
                        decG = gatp.tile([128, 1, IN_D], F32)
                        nc.gpsimd.dma_gather(
                            out_ap=decG,
                            in_ap=dec_d[:],
                            idxs_ap=wrap.bitcast(I16),
                            num_idxs=128,
                            num_idxs_reg=128,
                            elem_size=IN_D,
                        )
                        nc.gpsimd.dma_start(
                            out=bass.AP(
                                tensor=dec_o,
                                offset=g * 128 * IN_D,
                                ap=[[IN_D, 8], [8 * IN_D, 16], [1, IN_D]],
                            ),
                            in_=decG,
                        )
                        cbG = gatp.tile([128, 1, EMB_D], F32, tag="cbG")
                        nc.gpsimd.dma_gather(
                            out_ap=cbG,
                            in_ap=cb_p[:],
                            idxs_ap=wrap.bitcast(I16),
                            num_idxs=128,
                            num_idxs_reg=128,
                            elem_size=EMB_D,
                        )
                        nc.gpsimd.dma_start(
                            out=bass.AP(
                                tensor=quant_o,
                                offset=g * 128 * EMB_D,
                                ap=[[EMB_D, 8], [8 * EMB_D, 16], [1, EMB_D]],
                            ),
                            in_=cbG,
                        )
                nc.gpsimd.dma_start(out=gaps_o[:], in_=gap)
    nc.finalize()
    return nc


_NC_CACHE = None


def _get_nc():
    global _NC_CACHE
    if _NC_CACHE is None:
        _NC_CACHE = build()
    return _NC_CACHE


def kernel(x, W1, b1, W2, b2, codebook, W3, b3, W4, b4):
    x = np.ascontiguousarray(np.asarray(x, np.float32))
    W1 = np.asarray(W1, np.float32)
    b1 = np.asarray(b1, np.float32)
    W2 = np.asarray(W2, np.float32)
    b2 = np.asarray(b2, np.float32)
    cb = np.ascontiguousarray(np.asarray(codebook, np.float32))
    W3 = np.asarray(W3, np.float32)
    b3 = np.asarray(b3, np.float32)
    W4 = np.asarray(W4, np.float32)
    b4 = np.asarray(b4, np.float32)

    cbsq = (cb.astype(np.float64) ** 2).sum(-1).astype(np.float32)
    common = dict(
        W1r=r32r(W1),
        W2r=r32r(W2),
        W3r=r32r(W3),
        W4r=r32r(W4),
        cbTr=r32r(np.ascontiguousarray(cb.T)),
        cb=cb,
        cbsqh=np.ascontiguousarray(np.broadcast_to(0.5 * cbsq, (128, K))),
        b1l=np.ascontiguousarray(b1.reshape(16, 128).T),
        b2l=np.ascontiguousarray(b2.reshape(2, 128).T),
        b3l=np.ascontiguousarray(b3.reshape(16, 128).T),
        b4r=r32r(b4)[None, :],
        ones1=np.ones((1, 128), np.float32),
        ident=np.eye(128, dtype=np.float32),
    )
    in_maps = [
        dict(common, x=x[i * NTOK : (i + 1) * NTOK]) for i in range(NCORES)
    ]
    nc = _get_nc()
    res = run_bass_kernel_spmd(nc, in_maps, core_ids=list(range(NCORES)))

    quant = np.concatenate([res.results[i]["quant"] for i in range(NCORES)], 0)
    dec = np.concatenate([res.results[i]["dec"] for i in range(NCORES)], 0)
    gaps = np.concatenate(
        [
            res.results[i]["gaps"].T.reshape(-1)  # [128, NGRP] -> token order
            for i in range(NCORES)
        ]
    )

    # Host fixup: recompute tokens whose top-2 VQ score gap is tiny enough
    # that fp32r rounding could have flipped the argmin.
    flagged = np.nonzero(gaps < GAP_THRESHOLD)[0]
    if flagged.size:
        xf = x[flagged]
        h = np.maximum(xf @ W1 + b1, 0.0)
        e = h @ W2 + b2
        d2 = (e * e).sum(-1, keepdims=True) - 2.0 * (e @ cb.T) + cbsq
        idx = np.argmin(d2, axis=-1)
        q = cb[idx]
        h2 = np.maximum(q @ W3 + b3, 0.0)
        df = h2 @ W4 + b4
        quant[flagged] = q
        dec[flagged] = df
    return quant, dec


# revision 24
# speedup vs baseline: 1.7645x; 1.0329x over previous
"""DVQVAE (encoder -> VQ codebook argmin -> decoder) Trainium2 Bass kernel.

Strategy (8 NeuronCores, data-parallel over the 65536 tokens):
  - Each core processes 8192 tokens with all weights replicated.
  - All matmuls run as float32r (fp32 with 11-bit mantissa, full PE rate).
  - The decoder is evaluated once per CODEBOOK ENTRY (4096 rows), not per
    token: decoded rows are then fetched by index with indirect DMA
    (dma_gather), which removes ~80% of the decoder FLOPs.
  - Quantized rows are gathered from the raw fp32 codebook (bit-exact).
  - fp32r rounding can flip the VQ argmin when the top-2 score gap is tiny;
    the device also returns that gap and the host recomputes the ~1% of
    tokens whose gap is below a safety threshold in full fp32.

Layout notes: activations live feature-on-partition (xT/hT/eT) so layers
chain on the PE without transposes; x is transposed on-chip via the PE
transpose path. VQ scores are token-on-partition so the DVE max/max_index
top-8 instructions produce the argmin along the free axis.
"""
import sys

sys.path.insert(0, "/opt/trn_rl_repo")

import numpy as np
import concourse.bass as bass
import concourse.bacc as bacc
import concourse.mybir as mybir
import concourse.tile as tile
from concourse.bass_utils import run_bass_kernel_spmd

dt = mybir.dt
F32, F32R, U16, I16 = dt.float32, dt.float32r, dt.uint16, dt.int16
AF = mybir.ActivationFunctionType

N = 65536
NCORES = 8
NTOK = N // NCORES          # 8192 tokens per core
CHUNK = 256                 # tokens per main-loop chunk
NCHUNK = NTOK // CHUNK      # 32
NGRP = NTOK // 128          # 64 groups of 128 tokens
IN_D, HID_D, EMB_D, K = 1024, 2048, 256, 4096

GAP_THRESHOLD = 0.02        # host fixup flag: top-2 score gap below this


def r32r(a):
    """Round fp32 -> float32r bit pattern (11-bit mantissa, low 12 bits 0)."""
    u = np.ascontiguousarray(a, np.float32).view(np.uint32)
    return ((u + 0x800) & 0xFFFFF000).view(np.float32)


def build():
    nc = bacc.Bacc(None)

    x_p = nc.declare_dram_parameter("x", [NTOK, IN_D], F32, isOutput=False)
    W1_p = nc.declare_dram_parameter("W1r", [IN_D, HID_D], F32R, isOutput=False)
    W2_p = nc.declare_dram_parameter("W2r", [HID_D, EMB_D], F32R, isOutput=False)
    W3_p = nc.declare_dram_parameter("W3r", [EMB_D, HID_D], F32R, isOutput=False)
    W4_p = nc.declare_dram_parameter("W4r", [HID_D, IN_D], F32R, isOutput=False)
    cbT_p = nc.declare_dram_parameter("cbTr", [EMB_D, K], F32R, isOutput=False)
    cb_p = nc.declare_dram_parameter("cb", [K, EMB_D], F32, isOutput=False)
    cbsqh_p = nc.declare_dram_parameter("cbsqh", [128, K], F32, isOutput=False)
    b1_p = nc.declare_dram_parameter("b1l", [128, 16], F32, isOutput=False)
    b2_p = nc.declare_dram_parameter("b2l", [128, 2], F32, isOutput=False)
    b3_p = nc.declare_dram_parameter("b3l", [128, 16], F32, isOutput=False)
    b4_p = nc.declare_dram_parameter("b4r", [1, IN_D], F32R, isOutput=False)
    ones_p = nc.declare_dram_parameter("ones1", [1, 128], F32R, isOutput=False)
    id_p = nc.declare_dram_parameter("ident", [128, 128], F32, isOutput=False)

    quant_o = nc.declare_dram_parameter("quant", [NTOK, EMB_D], F32, isOutput=True)
    dec_o = nc.declare_dram_parameter("dec", [NTOK, IN_D], F32, isOutput=True)
    gaps_o = nc.declare_dram_parameter("gaps", [128, NGRP], F32, isOutput=True)
    idxs_o = nc.declare_dram_parameter("idxs", [128, NGRP], U16, isOutput=True)
    import os
    dbg = os.environ.get("BASSDBG") == "1"
    if dbg:
        dxT_o = nc.declare_dram_parameter("dxT", [128, 8, CHUNK], F32, isOutput=True)
        dhT_o = nc.declare_dram_parameter("dhT", [128, CHUNK], F32, isOutput=True)
        deT_o = nc.declare_dram_parameter("deT", [128, 2, CHUNK], F32, isOutput=True)
        dsc_o = nc.declare_dram_parameter("dsc", [128, K], F32, isOutput=True)

    dec_d = nc.dram_tensor("dec_d", [K, IN_D], F32)       # decoded-per-code table
    wd = nc.dram_tensor("wd_idx", [NCHUNK * CHUNK], U16)  # idx bounce buffer

    with tile.TileContext(nc) as tc:
        pconst = tc.tile_pool(name="pconst", bufs=1)
        with pconst as constp:
            cbT = constp.tile([128, 2, K], F32R)
            cbsq = constp.tile([128, K], F32)
            b1 = constp.tile([128, 16], F32)
            b2 = constp.tile([128, 2], F32)
            b3 = constp.tile([128, 16], F32)
            ident = constp.tile([128, 128], F32)
            for e in range(2):
                nc.sync.dma_start(out=cbT[:, e, :], in_=cbT_p[e * 128 : (e + 1) * 128, :])
            nc.sync.dma_start(out=cbsq, in_=cbsqh_p[:])
            nc.sync.dma_start(out=b1, in_=b1_p[:])
            nc.sync.dma_start(out=b2, in_=b2_p[:])
            nc.sync.dma_start(out=b3, in_=b3_p[:])
            nc.sync.dma_start(out=ident, in_=id_p[:])

            # ---- Phase 0: decoder table over the 4096 codebook rows ----
            with (
                tc.tile_pool(name="pw34", bufs=1) as w34p,
                tc.tile_pool(name="ptbl", bufs=1) as tblp,
                tc.tile_pool(name="pdec", bufs=2) as decp,
                tc.tile_pool(name="ps_h2", bufs=2, space="PSUM") as ps_h2,
                tc.tile_pool(name="ps_dec", bufs=2, space="PSUM") as ps_dec,
            ):
                W3 = w34p.tile([128, 2, HID_D], F32R)
                W4 = w34p.tile([128, 16, IN_D], F32R)
                b4 = w34p.tile([1, IN_D], F32R)
                ones = w34p.tile([1, 128], F32R)
                nc.sync.dma_start(out=b4, in_=b4_p[:])
                nc.sync.dma_start(out=ones, in_=ones_p[:])
                for k in range(2):
                    nc.sync.dma_start(out=W3[:, k, :], in_=W3_p[k * 128 : (k + 1) * 128, :])
                for k in range(16):
                    nc.sync.dma_start(out=W4[:, k, :], in_=W4_p[k * 128 : (k + 1) * 128, :])

                for cc in range(8):  # 512 codes per iteration
                    h2T = tblp.tile([128, 16, 512], F32R)
                    for m in range(16):
                        ph = ps_h2.tile([128, 512], F32)
                        for k in range(2):
                            nc.tensor.matmul(
                                ph,
                                W3[:, k, m * 128 : (m + 1) * 128],
                                cbT[:, k, cc * 512 : (cc + 1) * 512],
                                start=(k == 0),
                                stop=(k == 1),
                            )
                        nc.scalar.activation(h2T[:, m, :], ph, AF.Relu, bias=b3[:, m : m + 1])
                    for cs in range(4):  # 128-code subtiles -> code-major rows
                        decs = decp.tile([128, IN_D], F32)
                        for n in range(2):
                            pd = ps_dec.tile([128, 512], F32)
                            nc.tensor.matmul(
                                pd, ones, b4[:, n * 512 : (n + 1) * 512], start=True, stop=False
                            )
                            for k in range(16):
                                nc.tensor.matmul(
                                    pd,
                                    h2T[:, k, cs * 128 : (cs + 1) * 128],
                                    W4[:, k, n * 512 : (n + 1) * 512],
                                    start=False,
                                    stop=(k == 15),
                                )
                            nc.scalar.activation(decs[:, n * 512 : (n + 1) * 512], pd, AF.Copy)
                        nc.sync.dma_start(
                            out=dec_d[cc * 512 + cs * 128 : cc * 512 + (cs + 1) * 128, :],
                            in_=decs,
                        )

            # ---- Main loop: encoder + VQ + gathers, 256 tokens/chunk ----
            with (
                tc.tile_pool(name="pw12", bufs=1) as w12p,
                tc.tile_pool(name="pmain", bufs=1) as mp,
                tc.tile_pool(name="px", bufs=2) as xp,
                tc.tile_pool(name="pht", bufs=2) as htp,
                tc.tile_pool(name="pxt", bufs=1) as xtp,
                tc.tile_pool(name="pet", bufs=2) as etp,
                tc.tile_pool(name="pm8", bufs=4) as m8p,
                tc.tile_pool(name="pscore", bufs=2) as scp,
                tc.tile_pool(name="pidx", bufs=4) as idxp,
                tc.tile_pool(name="pwrap", bufs=4) as wrapp,
                tc.tile_pool(name="pgat", bufs=2) as gatp,
                tc.tile_pool(name="ps_mix", bufs=4, space="PSUM") as ps_mix,
                tc.tile_pool(name="ps_l1", bufs=2, space="PSUM") as ps_l1,
                tc.tile_pool(name="ps_et", bufs=1, space="PSUM") as ps_et,
            ):
                W1 = w12p.tile([128, 8, HID_D], F32R)
                W2 = w12p.tile([128, 16, EMB_D], F32R)
                for k in range(8):
                    nc.sync.dma_start(out=W1[:, k, :], in_=W1_p[k * 128 : (k + 1) * 128, :])
                for k in range(16):
                    nc.sync.dma_start(out=W2[:, k, :], in_=W2_p[k * 128 : (k + 1) * 128, :])

                gap = mp.tile([128, NGRP], F32)

                for c in range(NCHUNK):
                    xr = xp.tile([128, 2, IN_D], F32)
                    nc.sync.dma_start(
                        out=xr,
                        in_=x_p[c * CHUNK : (c + 1) * CHUNK, :].rearrange(
                            "(tp p) f -> p tp f", p=128
                        ),
                    )
                    xT = xtp.tile([128, 8, CHUNK], F32R, tag="xT")
                    for tp in range(2):
                        for kq in range(2):  # 4 packed transposes per PSUM bank
                            pt = ps_mix.tile([128, 4, 128], F32, tag="mix")
                            for ki in range(4):
                                k = kq * 4 + ki
                                nc.tensor.transpose(
                                    pt[:, ki, :], xr[:, tp, k * 128 : (k + 1) * 128], ident
                                )
                            nc.scalar.activation(
                                xT[:, kq * 4 : (kq + 1) * 4, tp * 128 : (tp + 1) * 128],
                                pt,
                                AF.Copy,
                            )
                    pe_ps0 = ps_et.tile([128, CHUNK], F32, tag="et0")
                    pe_ps1 = ps_et.tile([128, CHUNK], F32, tag="et1")
                    pe_ps = [pe_ps0, pe_ps1]
                    for m in range(16):
                        pl = ps_l1.tile([128, CHUNK], F32)
                        for k in range(8):
                            nc.tensor.matmul(
                                pl,
                                W1[:, k, m * 128 : (m + 1) * 128],
                                xT[:, k, :],
                                start=(k == 0),
                                stop=(k == 7),
                            )
                        hT = htp.tile([128, CHUNK], F32R)
                        nc.scalar.activation(hT, pl, AF.Relu, bias=b1[:, m : m + 1])
                        if dbg and c == 0 and m == 0:
                            nc.sync.dma_start(out=dhT_o[:], in_=hT.bitcast(F32))
                        for e in range(2):
                            nc.tensor.matmul(
                                pe_ps[e],
                                W2[:, m, e * 128 : (e + 1) * 128],
                                hT,
                                start=(m == 0),
                                stop=(m == 15),
                            )
                    eT = etp.tile([128, 2, CHUNK], F32R)
                    for e in range(2):
                        nc.scalar.activation(
                            eT[:, e, :], pe_ps[e], AF.Identity, bias=b2[:, e : e + 1]
                        )
                    if dbg and c == 0:
                        nc.sync.dma_start(out=dxT_o[:], in_=xT.bitcast(F32))
                        nc.sync.dma_start(out=deT_o[:], in_=eT.bitcast(F32))
                    # all 16 MM+subtract first so the DVE's long max/find ops
                    # never sit between the PE's VQ matmuls and their psum frees
                    scores = []
                    for j in range(2):  # two 128-token groups per chunk
                        score = scp.tile([128, K], F32, tag="score")
                        scores.append(score)
                        for cc in range(8):
                            pv = ps_mix.tile([128, 512], F32, tag="mix")
                            for e in range(2):
                                nc.tensor.matmul(
                                    pv,
                                    eT[:, e, j * 128 : (j + 1) * 128],
                                    cbT[:, e, cc * 512 : (cc + 1) * 512],
                                    start=(e == 0),
                                    stop=(e == 1),
                                )
                            nc.vector.tensor_sub(
                                score[:, cc * 512 : (cc + 1) * 512],
                                pv,
                                cbsq[:, cc * 512 : (cc + 1) * 512],
                            )
                    if dbg and c == 0:
                        nc.sync.dma_start(out=dsc_o[:], in_=scores[0])
                    wd_gp = wd[:].rearrange("(g p) -> g p", p=128)
                    for j in range(2):
                        g = 2 * c + j
                        m8 = m8p.tile([128, 8], F32)
                        i8 = m8p.tile([128, 8], U16, tag="i8")
                        nc.vector.max(m8, scores[j])
                        nc.vector.max_index(i8, m8, scores[j])
                        nc.vector.tensor_sub(
                            gap[:, g : g + 1], m8[:, 0:1], m8[:, 1:2]
                        )
                        idx16 = idxp.tile([128, 1], U16)
                        nc.vector.tensor_copy(idx16, i8[:, 0:1])
                        nc.gpsimd.dma_start(out=idxs_o[:, g : g + 1], in_=idx16)
                        # idx bounce: p-major dump, then wrapped+replicated read
                        nc.gpsimd.dma_start(out=wd_gp[g], in_=idx16[:, 0])
                        wrap = wrapp.tile([128, 8], U16)
                        nc.gpsimd.dma_start(
                            out=wrap,
                            in_=bass.AP(
                                tensor=wd,
                                offset=g * 128,
                                ap=[[0, 8], [8, 16], [1, 8]],
                            ),
                        )
                        # gather slot p' holds token g*128 + (p'%16)*8 + p'//16
                        decG = gatp.tile([128, 1, IN_D], F32)
                        nc.gpsimd.dma_gather(
                            out_ap=decG,
                            in_ap=dec_d[:],
                            idxs_ap=wrap.bitcast(I16),
                            num_idxs=128,
                            num_idxs_reg=128,
                            elem_size=IN_D,
                        )
                        nc.gpsimd.dma_start(
                            out=bass.AP(
                                tensor=dec_o,
                                offset=g * 128 * IN_D,
                                ap=[[IN_D, 8], [8 * IN_D, 16], [1, IN_D]],
                            ),
                            in_=decG,
                        )
                        cbG = gatp.tile([128, 1, EMB_D], F32, tag="cbG")
                        nc.gpsimd.dma_gather(
                            out_ap=cbG,
                            in_ap=cb_p[:],
                            idxs_ap=wrap.bitcast(I16),
                            num_idxs=128,
                            num_idxs_reg=128,
                            elem_size=EMB_D,
                        )
                        nc.gpsimd.dma_start(
                            out=bass.AP(
                                tensor=quant_o,
                                offset=g * 128 * EMB_D,
                                ap=[[EMB_D, 8], [8 * EMB_D, 16], [1, EMB_D]],
                            ),
                            in_=cbG,
                        )
                nc.gpsimd.dma_start(out=gaps_o[:], in_=gap)
    nc.finalize()
    return nc


_NC_CACHE = None


def _get_nc():
    global _NC_CACHE
    if _NC_CACHE is None:
        _NC_CACHE = build()
    return _NC_CACHE


def kernel(x, W1, b1, W2, b2, codebook, W3, b3, W4, b4):
    x = np.ascontiguousarray(np.asarray(x, np.float32))
    W1 = np.asarray(W1, np.float32)
    b1 = np.asarray(b1, np.float32)
    W2 = np.asarray(W2, np.float32)
    b2 = np.asarray(b2, np.float32)
    cb = np.ascontiguousarray(np.asarray(codebook, np.float32))
    W3 = np.asarray(W3, np.float32)
    b3 = np.asarray(b3, np.float32)
    W4 = np.asarray(W4, np.float32)
    b4 = np.asarray(b4, np.float32)

    cbsq = (cb.astype(np.float64) ** 2).sum(-1).astype(np.float32)
    common = dict(
        W1r=r32r(W1),
        W2r=r32r(W2),
        W3r=r32r(W3),
        W4r=r32r(W4),
        cbTr=r32r(np.ascontiguousarray(cb.T)),
        cb=cb,
        cbsqh=np.ascontiguousarray(np.broadcast_to(0.5 * cbsq, (128, K))),
        b1l=np.ascontiguousarray(b1.reshape(16, 128).T),
        b2l=np.ascontiguousarray(b2.reshape(2, 128).T),
        b3l=np.ascontiguousarray(b3.reshape(16, 128).T),
        b4r=r32r(b4)[None, :],
        ones1=np.ones((1, 128), np.float32),
        ident=np.eye(128, dtype=np.float32),
    )
    in_maps = [
        dict(common, x=x[i * NTOK : (i + 1) * NTOK]) for i in range(NCORES)
    ]
    nc = _get_nc()
    res = run_bass_kernel_spmd(nc, in_maps, core_ids=list(range(NCORES)))

    quant = np.concatenate([res.results[i]["quant"] for i in range(NCORES)], 0)
    dec = np.concatenate([res.results[i]["dec"] for i in range(NCORES)], 0)
    gaps = np.concatenate(
        [
            res.results[i]["gaps"].T.reshape(-1)  # [128, NGRP] -> token order
            for i in range(NCORES)
        ]
    )

    # Host fixup: recompute tokens whose top-2 VQ score gap is tiny enough
    # that fp32r rounding could have flipped the argmin.
    flagged = np.nonzero(gaps < GAP_THRESHOLD)[0]
    if flagged.size:
        xf = x[flagged]
        h = np.maximum(xf @ W1 + b1, 0.0)
        e = h @ W2 + b2
        d2 = (e * e).sum(-1, keepdims=True) - 2.0 * (e @ cb.T) + cbsq
        idx = np.argmin(d2, axis=-1)
        q = cb[idx]
        h2 = np.maximum(q @ W3 + b3, 0.0)
        df = h2 @ W4 + b4
        quant[flagged] = q
        dec[flagged] = df
    return quant, dec
